# revision 38
# baseline (speedup 1.0000x reference)
"""Multi-head self-attention (B=2, S=2048, E=1024, H=16, D=64, causal) on 8 trn2 cores.

Sharding: tensor-parallel over (batch, head-group). Core c handles batch c//4 and
heads [4*(c%4), 4*(c%4)+4). Each core computes QKV projection for its 4 heads,
causal flash-attention, and a partial output projection (its heads' rows of
w_out). Host sums the 4 partials per batch and adds b_out.

Device math (per core, bf16 matmuls):
  qT/kT [j, s] = (wqk_ext).T @ xT_ext     (j on partitions -> scores need no transpose)
  v_ext [s, j] = xT_ext.T @ wv_ext        (per head: [v|ones] or [ones|v] 128-col block)
  S^T tile [sk, sq] = kT.T-slice @ qT-slice  (two heads row-tiled on the PE)
  P^T = exp(S^T / 8) with causal triangle mask; no max-subtraction needed
  PV: [O^T; L] = v_ext.T @ P^T accumulated over sk chunks; L = softmax denominator
  O^T normalized by 1/L, projected: out_partial = OT.T @ wout_rows

Engine assignment (vs the naive version): ACT (scalar) runs ONLY exp; all
psum->sbuf cast copies run on DVE (vector); causal mask multiplies run on
GpSimd; outputs are written bf16. The out-projection is interleaved into the
attention phase as PE fill work, and the PE/ACT/GPSIMD pipelines are warmed
during the initial DMA window.
"""
import sys

sys.path.insert(0, "/opt/trn_rl_repo")

import ml_dtypes
import numpy as np

import concourse.bacc as bacc
import concourse.mybir as mybir
import concourse.tile as tile



B, S, E = 2, 2048, 1024
H, D = 16, 64
HPC = 4          # heads per core
NCORES = 8
SC = 512         # sq chunk width (scores free dim)
KC = 128         # sk chunk width
NQC = S // SC    # 4 q-chunks
NSB = S // 128   # 16 s-blocks

f32 = mybir.dt.float32
bf16 = mybir.dt.bfloat16

_NC = None


def _build_nc():
    nc = bacc.Bacc(None, target_bir_lowering=False)

    xT = nc.dram_tensor("xT", [E, S], bf16, kind="ExternalInput")
    wqk = nc.dram_tensor("wqk", [E, 512], bf16, kind="ExternalInput")
    wv = nc.dram_tensor("wv", [E, 256], bf16, kind="ExternalInput")
    wout = nc.dram_tensor("wout", [256, E], bf16, kind="ExternalInput")
    mask = nc.dram_tensor("mask", [128, 128], bf16, kind="ExternalInput")
    out_p = nc.dram_tensor("out_p", [S, E], bf16, kind="ExternalOutput")

    with tile.TileContext(nc) as tc:
        with (
            tc.tile_pool(name="big", bufs=1) as big,
            tc.tile_pool(name="ptp", bufs=6) as ptp,
            tc.tile_pool(name="lvp", bufs=2) as lvp,
            tc.tile_pool(name="osb", bufs=3) as osbp,
            tc.tile_pool(name="psA", bufs=2, space="PSUM") as psA,
            tc.tile_pool(name="psB", bufs=1, space="PSUM") as psB,
            tc.tile_pool(name="psF", bufs=2, space="PSUM") as psF,
        ):
            xT_sb = big.tile([128, 8, S], bf16)
            wqk_sb = big.tile([128, 8, 512], bf16)
            wv_sb = big.tile([128, 8, 256], bf16)
            qkT_sb = big.tile([128, 4, S], bf16)
            v_sb = big.tile([128, NSB, 512], bf16)
            OT_sb = big.tile([128, 2, S], bf16)
            wout_sb = big.tile([128, 2, E], bf16)
            mask_sb = big.tile([128, 128], bf16)
            scr_sb = big.tile([128, 128], bf16)   # warmup scratch output

            # ---- engine warmup: ACT table load, GPSIMD library load, PE HAM ----
            # mask is the first (tiny) DMA; everything below depends only on it
            # and executes during the bulk input-DMA window.
            nc.sync.dma_start(out=mask_sb, in_=mask[:, :])
            nc.scalar.activation(
                out=scr_sb[:, 0:8], in_=mask_sb[:, 0:8],
                func=mybir.ActivationFunctionType.Exp, scale=0.125)
            nc.gpsimd.tensor_mul(scr_sb[:, 0:8], mask_sb[:, 0:8], mask_sb[:, 0:8])
            wps = psF.tile([128, SC], f32, name="ps512")
            for _ in range(36):
                nc.tensor.matmul(wps[:, 0:128], mask_sb[:, :], mask_sb[:, :],
                                 start=True, stop=True)

            # ones columns of v_ext: [64:192] and [320:448] within each 512
            # block -- constants, no DMA needed.
            nc.vector.memset(v_sb[:, :, 64:192], 1.0)
            nc.vector.memset(v_sb[:, :, 320:448], 1.0)

            # ---- input DMAs, ordered by first use ----
            # One DMA instruction per logical chunk (descriptor generation on
            # the SP sequencer is ~0.6us per instruction -- many small DMAs
            # starve the prologue). wqk columns: jb0 = q heads 0,1 / jb1 = q
            # heads 2,3 / jb2 = k heads 0,1 / jb3 = k heads 2,3.
            def dma_cols(dst, src_cols):
                nc.sync.dma_start(
                    out=dst, in_=src_cols.rearrange("(b p) c -> p b c", p=128))

            def dma_cols_rows(dst, src_cols, k0, k1):
                nc.sync.dma_start(
                    out=dst[:, k0:k1],
                    in_=src_cols[k0 * 128:k1 * 128, :].rearrange(
                        "(b p) c -> p b c", p=128))

            # first chunk in halves so the prologue matmuls start sooner
            dma_cols_rows(wqk_sb[:, :, 0:128], wqk[:, 0:128], 0, 4)
            dma_cols_rows(xT_sb[:, :, 0:SC], xT[:, 0:SC], 0, 4)
            dma_cols_rows(wqk_sb[:, :, 0:128], wqk[:, 0:128], 4, 8)
            dma_cols_rows(xT_sb[:, :, 0:SC], xT[:, 0:SC], 4, 8)
            dma_cols(wqk_sb[:, :, 256:384], wqk[:, 256:384])
            dma_cols(wv_sb[:, :, :], wv[:, :])
            dma_cols(wqk_sb[:, :, 128:256], wqk[:, 128:256])
            dma_cols(wqk_sb[:, :, 384:512], wqk[:, 384:512])
            for sc4 in range(1, 4):
                dma_cols(xT_sb[:, :, sc4 * SC:(sc4 + 1) * SC],
                         xT[:, sc4 * SC:(sc4 + 1) * SC])
            nc.sync.dma_start(
                out=wout_sb[:, :, :], in_=wout[:, :].rearrange("(b p) c -> p b c", p=128))

            # ---- QKV projection fill units (half = 4 of 8 k-chunks) ----
            def make_qk_fill(jb, sc):
                state = {}

                def half(h):
                    if h == 0:
                        state["ps"] = psF.tile([128, SC], f32, name="ps512")
                    ps = state["ps"]
                    for kc in range(4 * h, 4 * h + 4):
                        nc.tensor.matmul(
                            ps[:, :],
                            wqk_sb[:, kc, jb * 128:(jb + 1) * 128],
                            xT_sb[:, kc, sc * SC:(sc + 1) * SC],
                            start=(kc == 0), stop=(kc == 7))
                    if h == 1:
                        nc.vector.tensor_copy(
                            out=qkT_sb[:, jb, sc * SC:(sc + 1) * SC], in_=ps[:, :])

                return [lambda: half(0), lambda: half(1)]

            def make_v_fill(sb):
                def go():
                    # raw v [128, 256] = xT[:, sb*128:+128].T @ wv; heads h0..h3.
                    # v_ext per head pair: [v_e | ones | ones | v_o].
                    ps = psF.tile([128, SC], f32, name="ps512")[:, 0:256]
                    for kc in range(8):
                        nc.tensor.matmul(
                            ps[:, :],
                            xT_sb[:, kc, sb * 128:(sb + 1) * 128],
                            wv_sb[:, kc, :],
                            start=(kc == 0), stop=(kc == 7))
                    ps3 = ps.rearrange("p (b c) -> p b c", c=128)
                    vs3 = v_sb[:, sb, :].rearrange("p (b c) -> p b c", c=256)
                    nc.vector.tensor_copy(out=vs3[:, :, 0:64], in_=ps3[:, :, 0:64])
                    nc.vector.tensor_copy(out=vs3[:, :, 192:256], in_=ps3[:, :, 64:128])
                return [go]

            def make_proj_fill(sc, nh, cast_on_scalar=False):
                def go():
                    po = psF.tile([128, SC], f32, name="ps512")
                    for p in range(2):
                        nc.tensor.matmul(
                            po[:, :],
                            OT_sb[:, p, sc * 128:(sc + 1) * 128],
                            wout_sb[:, p, SC * nh:SC * nh + SC],
                            start=(p == 0), stop=(p == 1))
                    osb = osbp.tile([128, SC], bf16, name="osb")
                    if cast_on_scalar:
                        nc.scalar.copy(out=osb, in_=po[:, :])
                    else:
                        nc.vector.tensor_copy(out=osb, in_=po[:, :])
                    nc.sync.dma_start(
                        out=out_p[sc * 128:(sc + 1) * 128, SC * nh:SC * nh + SC],
                        in_=osb)
                return [go]

            # ---- attention for one (head pair, q-chunk), with PE fill work ----
            def attention_qc(pair, qc, fills=(), split_norm=False):
                qblk, kblk = pair, 2 + pair
                nkc = 4 * qc + 4
                pv = psB.tile([128, 1024], f32, name="pv")
                pvh = [pv[:, 0:SC], pv[:, SC:1024]]

                def scores_exp(kc):
                    # diagonal tiles (r >= 0): columns < 128*r are causally
                    # invalid -- skip them in the matmul and PV (ragged).
                    r = kc - 4 * qc
                    off = KC * r if r > 0 else 0
                    st = psA.tile([128, 1024], f32, tag="A", name="st")
                    nc.tensor.matmul(
                        st[:, off:SC],
                        qkT_sb[0:64, kblk, kc * KC:(kc + 1) * KC],
                        qkT_sb[0:64, qblk, qc * SC + off:(qc + 1) * SC],
                        start=True, stop=True, tile_position=(0, 0))
                    nc.tensor.matmul(
                        st[:, SC + off:1024],
                        qkT_sb[64:128, kblk, kc * KC:(kc + 1) * KC],
                        qkT_sb[64:128, qblk, qc * SC + off:(qc + 1) * SC],
                        start=True, stop=True, tile_position=(64, 0))
                    pt = ptp.tile([128, 1024], bf16, name="pt")
                    if r < 0:
                        nc.scalar.activation(
                            out=pt[:, :], in_=st[:, :],
                            func=mybir.ActivationFunctionType.Exp, scale=0.125)
                    else:
                        # one wide exp where that costs fewer ACT cycles than two
                        # narrow ones; the gap [SC:SC+off] is junk and never read.
                        if r <= 2:
                            nc.scalar.activation(
                                out=pt[:, off:1024], in_=st[:, off:1024],
                                func=mybir.ActivationFunctionType.Exp, scale=0.125)
                        else:
                            for h2 in range(2):
                                base = SC * h2
                                nc.scalar.activation(
                                    out=pt[:, base + off:base + SC],
                                    in_=st[:, base + off:base + SC],
                                    func=mybir.ActivationFunctionType.Exp, scale=0.125)
                        for h2 in range(2):
                            base = SC * h2
                            tri = pt[:, base + off:base + off + KC]
                            nc.gpsimd.tensor_mul(tri, tri, mask_sb[:, :])
                    return pt

                def pv_step(kc, pt):
                    r = kc - 4 * qc
                    off = KC * r if r > 0 else 0
                    for h2 in range(2):
                        hh = 2 * pair + h2
                        nc.tensor.matmul(
                            pvh[h2][:, off:SC],
                            v_sb[:, kc, 128 * hh:128 * hh + 128],
                            pt[:, SC * h2 + off:SC * h2 + SC],
                            start=(kc == 0), stop=(kc == nkc - 1))

                # pv lags scores by 3 chunks so the pv LDWEIGHTS (which carries
                # the exp-done wait) is always ready and can be pulled ahead.
                lag = min(4, nkc - 1)
                fills = list(fills)
                fi = 0
                pts = {}
                for kc in range(nkc):
                    pts[kc] = scores_exp(kc)
                    if fi < len(fills):
                        fills[fi]()
                        fi += 1
                    if kc >= lag:
                        pv_step(kc - lag, pts.pop(kc - lag))
                while fi < len(fills):
                    fills[fi]()
                    fi += 1
                for kc in range(nkc - lag, nkc):
                    if kc >= 0 and kc in pts:
                        pv_step(kc, pts.pop(kc))

                # normalization: even head [v|ones] -> O rows 0:64 / L rows 64:128
                # of bank0; odd head [ones|v] -> L rows 0:64 / O rows 64:128 of
                # bank1. reciprocal_approx_fast is broken at base_partition != 0,
                # so read full 128 partitions (unused rows produce garbage that
                # is never consumed).
                # shuffle the L rows out of PSUM first (DVE shuffle crossbar;
                # identity mask -- the partition shift is encoded in the AP base
                # partitions), then one narrow reciprocal on SBUF. Keeps the
                # whole normalization chain on one engine queue.
                lr = lvp.tile([128, SC], f32, tag="rec", name="rec")
                ident = list(range(32))
                nc.vector.stream_shuffle(out=lr[0:64, :], in_=pvh[0][64:128, :],
                                         mask=ident)
                nc.vector.stream_shuffle(out=lr[64:128, :], in_=pvh[1][0:64, :],
                                         mask=ident)
                linv = lvp.tile([128, SC], f32, tag="linv", name="linv")
                nc.vector.reciprocal_approx_fast(out=linv[:, :], in_=lr[:, :])
                qs = qc * SC
                if split_norm:
                    # narrow chunks so the first epilogue projection (which
                    # needs only 128 OT columns) starts as early as possible
                    for c0 in range(0, SC, 128):
                        nc.vector.tensor_mul(
                            OT_sb[0:64, pair, qs + c0:qs + c0 + 128],
                            pvh[0][0:64, c0:c0 + 128], linv[0:64, c0:c0 + 128])
                        nc.vector.tensor_mul(
                            OT_sb[64:128, pair, qs + c0:qs + c0 + 128],
                            pvh[1][64:128, c0:c0 + 128], linv[64:128, c0:c0 + 128])
                else:
                    nc.vector.tensor_mul(
                        OT_sb[0:64, pair, qs:qs + SC], pvh[0][0:64, :], linv[0:64, :])
                    nc.vector.tensor_mul(
                        OT_sb[64:128, pair, qs:qs + SC], pvh[1][64:128, :], linv[64:128, :])

            # ---- emission schedule ----
            qk = {}
            for jb in range(4):
                for sc in range(4):
                    qk[(jb, sc)] = make_qk_fill(jb, sc)
            vfill = {sb: make_v_fill(sb)[0] for sb in range(NSB)}
            prj = {}
            for sc in range(NSB):
                for nh in range(2):
                    # epilogue chunks (after the last exp) alternate their cast
                    # between DVE and the then-idle ACT engine
                    prj[(sc, nh)] = make_proj_fill(
                        sc, nh, cast_on_scalar=(sc >= 12 and nh == 1))[0]

            # prologue: q+k projections for (pair0, qc0)
            for f in qk[(0, 0)] + qk[(2, 0)]:
                f()

            attention_qc(0, 0, fills=[
                vfill[0], vfill[1], vfill[2], vfill[3],
                *qk[(1, 0)], *qk[(3, 0)]])
            attention_qc(1, 0, fills=[*qk[(0, 1)], *qk[(2, 1)]])
            attention_qc(0, 1, fills=[
                vfill[4], vfill[5], vfill[6], vfill[7],
                *qk[(1, 1)], *qk[(3, 1)]])
            attention_qc(1, 1, fills=[
                *qk[(0, 2)], *qk[(2, 2)],
                prj[(0, 0)], prj[(0, 1)], prj[(1, 0)], prj[(1, 1)]])
            attention_qc(0, 2, fills=[
                vfill[8], vfill[9], vfill[10], vfill[11],
                *qk[(1, 2)], *qk[(3, 2)],
                prj[(2, 0)], prj[(2, 1)], prj[(3, 0)], prj[(3, 1)]])
            attention_qc(1, 2, fills=[
                *qk[(0, 3)], *qk[(2, 3)],
                prj[(4, 0)], prj[(4, 1)], prj[(5, 0)], prj[(5, 1)]])
            attention_qc(0, 3, fills=[
                vfill[12], vfill[13], vfill[14], vfill[15],
                *qk[(1, 3)], *qk[(3, 3)],
                prj[(6, 0)], prj[(6, 1)], prj[(7, 0)], prj[(7, 1)]])
            attention_qc(1, 3, fills=[
                prj[(8, 0)], prj[(8, 1)], prj[(9, 0)], prj[(9, 1)],
                prj[(10, 0)], prj[(10, 1)], prj[(11, 0)], prj[(11, 1)]],
                split_norm=True)
            for sc in range(12, 16):
                prj[(sc, 0)]()
                prj[(sc, 1)]()

    nc.finalize()
    return nc


def _get_nc():
    global _NC
    if _NC is None:
        _NC = _build_nc()
    return _NC


def _prep_in_maps(x, w_qkv, b_qkv):
    x = np.asarray(x, dtype=np.float32)
    w_qkv = np.asarray(w_qkv, dtype=np.float32)
    b_qkv = np.asarray(b_qkv, dtype=np.float32)

    xT_by_batch = [np.ascontiguousarray(x[b].T).astype(ml_dtypes.bfloat16) for b in range(B)]

    mask = np.triu(np.ones((128, 128), dtype=ml_dtypes.bfloat16))  # valid where sq >= sk

    in_maps = []
    for c in range(NCORES):
        b, g = divmod(c, HPC)
        h0 = HPC * g  # first global head for this core
        cq = slice(h0 * D, (h0 + HPC) * D)
        ck = slice(H * D + h0 * D, H * D + (h0 + HPC) * D)

        wqk = np.empty((E, 512), dtype=ml_dtypes.bfloat16)
        wqk[:, 0:256] = w_qkv[:, cq]
        wqk[:, 256:512] = w_qkv[:, ck]

        # b_qkv is zeros by the problem spec (fill: zeros); the device program
        # has no bias path.
        cv = slice(2 * H * D + h0 * D, 2 * H * D + (h0 + HPC) * D)
        wv = np.ascontiguousarray(w_qkv[:, cv]).astype(ml_dtypes.bfloat16)

        in_maps.append({
            "xT": xT_by_batch[b],
            "wqk": wqk,
            "wv": wv,
            "wout": None,  # filled by caller (needs w_out)
            "mask": mask,
        })
    return in_maps


def run(x, w_qkv, b_qkv, w_out, b_out, trace=False, **spmd_kwargs):
    from concourse.bass_utils import run_bass_kernel_spmd

    w_out = np.asarray(w_out, dtype=np.float32)
    b_out = np.asarray(b_out, dtype=np.float32)
    in_maps = _prep_in_maps(x, w_qkv, b_qkv)
    for c in range(NCORES):
        h0 = HPC * (c % HPC)
        in_maps[c]["wout"] = np.ascontiguousarray(w_out[h0 * D:(h0 + HPC) * D, :]).astype(ml_dtypes.bfloat16)

    nc = _get_nc()
    res = run_bass_kernel_spmd(nc, in_maps, core_ids=list(range(NCORES)),
                               trace=trace, **spmd_kwargs)
    out = np.empty((B, S, E), dtype=np.float32)
    for b in range(B):
        acc = res.results[HPC * b]["out_p"].astype(np.float32)
        for i in range(1, HPC):
            acc = acc + res.results[HPC * b + i]["out_p"].astype(np.float32)
        out[b] = acc + b_out
    return out, res


def kernel(x, w_qkv, b_qkv, w_out, b_out):
    out, _ = run(x, w_qkv, b_qkv, w_out, b_out, trace=False)
    return out


# revision 39
# speedup vs baseline: 1.0106x; 1.0106x over previous
"""Multi-head self-attention (B=2, S=2048, E=1024, H=16, D=64, causal) on 8 trn2 cores.

Sharding: tensor-parallel over (batch, head-group). Core c handles batch c//4 and
heads [4*(c%4), 4*(c%4)+4). Each core computes QKV projection for its 4 heads,
causal flash-attention, and a partial output projection (its heads' rows of
w_out). Host sums the 4 partials per batch and adds b_out.

Device math (per core, bf16 matmuls):
  qT/kT [j, s] = (wqk_ext).T @ xT_ext     (j on partitions -> scores need no transpose)
  v_ext [s, j] = xT_ext.T @ wv_ext        (per head: [v|ones] or [ones|v] 128-col block)
  S^T tile [sk, sq] = kT.T-slice @ qT-slice  (two heads row-tiled on the PE)
  P^T = exp(S^T / 8) with causal triangle mask; no max-subtraction needed
  PV: [O^T; L] = v_ext.T @ P^T accumulated over sk chunks; L = softmax denominator
  O^T normalized by 1/L, projected: out_partial = OT.T @ wout_rows

Engine assignment (vs the naive version): ACT (scalar) runs ONLY exp; all
psum->sbuf cast copies run on DVE (vector); causal mask multiplies run on
GpSimd; outputs are written bf16. The out-projection is interleaved into the
attention phase as PE fill work, and the PE/ACT/GPSIMD pipelines are warmed
during the initial DMA window.
"""
import sys

sys.path.insert(0, "/opt/trn_rl_repo")

import ml_dtypes
import numpy as np

import concourse.bacc as bacc
import concourse.mybir as mybir
import concourse.tile as tile



B, S, E = 2, 2048, 1024
H, D = 16, 64
HPC = 4          # heads per core
NCORES = 8
SC = 512         # sq chunk width (scores free dim)
KC = 128         # sk chunk width
NQC = S // SC    # 4 q-chunks
NSB = S // 128   # 16 s-blocks

f32 = mybir.dt.float32
bf16 = mybir.dt.bfloat16

_NC = None


def _build_nc():
    nc = bacc.Bacc(None, target_bir_lowering=False)

    xT = nc.dram_tensor("xT", [E, S], bf16, kind="ExternalInput")
    wqk = nc.dram_tensor("wqk", [E, 512], bf16, kind="ExternalInput")
    wv = nc.dram_tensor("wv", [E, 256], bf16, kind="ExternalInput")
    wout = nc.dram_tensor("wout", [256, E], bf16, kind="ExternalInput")
    mask = nc.dram_tensor("mask", [128, 128], bf16, kind="ExternalInput")
    out_p = nc.dram_tensor("out_p", [S, E], bf16, kind="ExternalOutput")

    with tile.TileContext(nc) as tc:
        with (
            tc.tile_pool(name="big", bufs=1) as big,
            tc.tile_pool(name="ptp", bufs=6) as ptp,
            tc.tile_pool(name="lvp", bufs=2) as lvp,
            tc.tile_pool(name="osb", bufs=3) as osbp,
            tc.tile_pool(name="psA", bufs=2, space="PSUM") as psA,
            tc.tile_pool(name="psB", bufs=1, space="PSUM") as psB,
            tc.tile_pool(name="psF", bufs=2, space="PSUM") as psF,
        ):
            xT_sb = big.tile([128, 8, S], bf16)
            wqk_sb = big.tile([128, 8, 512], bf16)
            wv_sb = big.tile([128, 8, 256], bf16)
            qkT_sb = big.tile([128, 4, S], bf16)
            v_sb = big.tile([128, NSB, 512], bf16)
            OT_sb = big.tile([128, 2, S], bf16)
            wout_sb = big.tile([128, 2, E], bf16)
            mask_sb = big.tile([128, 128], bf16)
            scr_sb = big.tile([128, 128], bf16)   # warmup scratch output

            # ---- engine warmup: ACT table load, GPSIMD library load, PE HAM ----
            # mask is the first (tiny) DMA; everything below depends only on it
            # and executes during the bulk input-DMA window.
            nc.sync.dma_start(out=mask_sb, in_=mask[:, :])
            nc.scalar.activation(
                out=scr_sb[:, 0:8], in_=mask_sb[:, 0:8],
                func=mybir.ActivationFunctionType.Exp, scale=0.125)
            nc.gpsimd.tensor_mul(scr_sb[:, 0:8], mask_sb[:, 0:8], mask_sb[:, 0:8])
            wps = psF.tile([128, SC], f32, name="ps512")
            for _ in range(52):
                nc.tensor.matmul(wps[:, 0:128], mask_sb[:, :], mask_sb[:, :],
                                 start=True, stop=True)

            # ones columns of v_ext: [64:192] and [320:448] within each 512
            # block -- constants, no DMA needed.
            nc.vector.memset(v_sb[:, :, 64:192], 1.0)
            nc.vector.memset(v_sb[:, :, 320:448], 1.0)

            # ---- input DMAs, ordered by first use ----
            # One DMA instruction per logical chunk (descriptor generation on
            # the SP sequencer is ~0.6us per instruction -- many small DMAs
            # starve the prologue). wqk columns: jb0 = q heads 0,1 / jb1 = q
            # heads 2,3 / jb2 = k heads 0,1 / jb3 = k heads 2,3.
            def dma_cols(dst, src_cols):
                nc.sync.dma_start(
                    out=dst, in_=src_cols.rearrange("(b p) c -> p b c", p=128))

            def dma_cols_rows(dst, src_cols, k0, k1):
                nc.sync.dma_start(
                    out=dst[:, k0:k1],
                    in_=src_cols[k0 * 128:k1 * 128, :].rearrange(
                        "(b p) c -> p b c", p=128))

            # first chunk in halves so the prologue matmuls start sooner
            dma_cols_rows(wqk_sb[:, :, 0:128], wqk[:, 0:128], 0, 4)
            dma_cols_rows(xT_sb[:, :, 0:SC], xT[:, 0:SC], 0, 4)
            dma_cols_rows(wqk_sb[:, :, 0:128], wqk[:, 0:128], 4, 8)
            dma_cols_rows(xT_sb[:, :, 0:SC], xT[:, 0:SC], 4, 8)
            dma_cols(wqk_sb[:, :, 256:384], wqk[:, 256:384])
            dma_cols(wv_sb[:, :, :], wv[:, :])
            dma_cols(wqk_sb[:, :, 128:256], wqk[:, 128:256])
            dma_cols(wqk_sb[:, :, 384:512], wqk[:, 384:512])
            for sc4 in range(1, 4):
                dma_cols(xT_sb[:, :, sc4 * SC:(sc4 + 1) * SC],
                         xT[:, sc4 * SC:(sc4 + 1) * SC])
            nc.sync.dma_start(
                out=wout_sb[:, :, :], in_=wout[:, :].rearrange("(b p) c -> p b c", p=128))

            # ---- QKV projection fill units (half = 4 of 8 k-chunks) ----
            def make_qk_fill(jb, sc):
                state = {}

                def half(h):
                    if h == 0:
                        state["ps"] = psF.tile([128, SC], f32, name="ps512")
                    ps = state["ps"]
                    for kc in range(4 * h, 4 * h + 4):
                        nc.tensor.matmul(
                            ps[:, :],
                            wqk_sb[:, kc, jb * 128:(jb + 1) * 128],
                            xT_sb[:, kc, sc * SC:(sc + 1) * SC],
                            start=(kc == 0), stop=(kc == 7))
                    if h == 1:
                        nc.vector.tensor_copy(
                            out=qkT_sb[:, jb, sc * SC:(sc + 1) * SC], in_=ps[:, :])

                return [lambda: half(0), lambda: half(1)]

            def make_v_fill(sb):
                def go():
                    # raw v [128, 256] = xT[:, sb*128:+128].T @ wv; heads h0..h3.
                    # v_ext per head pair: [v_e | ones | ones | v_o].
                    ps = psF.tile([128, SC], f32, name="ps512")[:, 0:256]
                    for kc in range(8):
                        nc.tensor.matmul(
                            ps[:, :],
                            xT_sb[:, kc, sb * 128:(sb + 1) * 128],
                            wv_sb[:, kc, :],
                            start=(kc == 0), stop=(kc == 7))
                    ps3 = ps.rearrange("p (b c) -> p b c", c=128)
                    vs3 = v_sb[:, sb, :].rearrange("p (b c) -> p b c", c=256)
                    nc.vector.tensor_copy(out=vs3[:, :, 0:64], in_=ps3[:, :, 0:64])
                    nc.vector.tensor_copy(out=vs3[:, :, 192:256], in_=ps3[:, :, 64:128])
                return [go]

            def make_proj_fill(sc, nh, cast_on_scalar=False):
                def go():
                    po = psF.tile([128, SC], f32, name="ps512")
                    for p in range(2):
                        nc.tensor.matmul(
                            po[:, :],
                            OT_sb[:, p, sc * 128:(sc + 1) * 128],
                            wout_sb[:, p, SC * nh:SC * nh + SC],
                            start=(p == 0), stop=(p == 1))
                    osb = osbp.tile([128, SC], bf16, name="osb")
                    if cast_on_scalar:
                        nc.scalar.copy(out=osb, in_=po[:, :])
                    else:
                        nc.vector.tensor_copy(out=osb, in_=po[:, :])
                    nc.sync.dma_start(
                        out=out_p[sc * 128:(sc + 1) * 128, SC * nh:SC * nh + SC],
                        in_=osb)
                return [go]

            # ---- attention for one (head pair, q-chunk), with PE fill work ----
            def attention_qc(pair, qc, fills=(), split_norm=False):
                qblk, kblk = pair, 2 + pair
                nkc = 4 * qc + 4
                pv = psB.tile([128, 1024], f32, name="pv")
                pvh = [pv[:, 0:SC], pv[:, SC:1024]]

                def scores_exp(kc):
                    # diagonal tiles (r >= 0): columns < 128*r are causally
                    # invalid -- skip them in the matmul and PV (ragged).
                    r = kc - 4 * qc
                    off = KC * r if r > 0 else 0
                    st = psA.tile([128, 1024], f32, tag="A", name="st")
                    nc.tensor.matmul(
                        st[:, off:SC],
                        qkT_sb[0:64, kblk, kc * KC:(kc + 1) * KC],
                        qkT_sb[0:64, qblk, qc * SC + off:(qc + 1) * SC],
                        start=True, stop=True, tile_position=(0, 0))
                    nc.tensor.matmul(
                        st[:, SC + off:1024],
                        qkT_sb[64:128, kblk, kc * KC:(kc + 1) * KC],
                        qkT_sb[64:128, qblk, qc * SC + off:(qc + 1) * SC],
                        start=True, stop=True, tile_position=(64, 0))
                    pt = ptp.tile([128, 1024], bf16, name="pt")
                    if r < 0:
                        nc.scalar.activation(
                            out=pt[:, :], in_=st[:, :],
                            func=mybir.ActivationFunctionType.Exp, scale=0.125)
                    else:
                        # one wide exp where that costs fewer ACT cycles than two
                        # narrow ones; the gap [SC:SC+off] is junk and never read.
                        if r <= 2:
                            nc.scalar.activation(
                                out=pt[:, off:1024], in_=st[:, off:1024],
                                func=mybir.ActivationFunctionType.Exp, scale=0.125)
                        else:
                            for h2 in range(2):
                                base = SC * h2
                                nc.scalar.activation(
                                    out=pt[:, base + off:base + SC],
                                    in_=st[:, base + off:base + SC],
                                    func=mybir.ActivationFunctionType.Exp, scale=0.125)
                        for h2 in range(2):
                            base = SC * h2
                            tri = pt[:, base + off:base + off + KC]
                            nc.gpsimd.tensor_mul(tri, tri, mask_sb[:, :])
                    return pt

                def pv_step(kc, pt):
                    r = kc - 4 * qc
                    off = KC * r if r > 0 else 0
                    for h2 in range(2):
                        hh = 2 * pair + h2
                        nc.tensor.matmul(
                            pvh[h2][:, off:SC],
                            v_sb[:, kc, 128 * hh:128 * hh + 128],
                            pt[:, SC * h2 + off:SC * h2 + SC],
                            start=(kc == 0), stop=(kc == nkc - 1))

                # pv lags scores by 3 chunks so the pv LDWEIGHTS (which carries
                # the exp-done wait) is always ready and can be pulled ahead.
                lag = min(4, nkc - 1)
                fills = list(fills)
                fi = 0
                pts = {}
                for kc in range(nkc):
                    pts[kc] = scores_exp(kc)
                    if fi < len(fills):
                        fills[fi]()
                        fi += 1
                    if kc >= lag:
                        pv_step(kc - lag, pts.pop(kc - lag))
                while fi < len(fills):
                    fills[fi]()
                    fi += 1
                for kc in range(nkc - lag, nkc):
                    if kc >= 0 and kc in pts:
                        pv_step(kc, pts.pop(kc))

                # normalization: even head [v|ones] -> O rows 0:64 / L rows 64:128
                # of bank0; odd head [ones|v] -> L rows 0:64 / O rows 64:128 of
                # bank1. reciprocal_approx_fast is broken at base_partition != 0,
                # so read full 128 partitions (unused rows produce garbage that
                # is never consumed).
                # shuffle the L rows out of PSUM first (DVE shuffle crossbar;
                # identity mask -- the partition shift is encoded in the AP base
                # partitions), then one narrow reciprocal on SBUF. Keeps the
                # whole normalization chain on one engine queue.
                lr = lvp.tile([128, SC], f32, tag="rec", name="rec")
                ident = list(range(32))
                nc.vector.stream_shuffle(out=lr[0:64, :], in_=pvh[0][64:128, :],
                                         mask=ident)
                nc.vector.stream_shuffle(out=lr[64:128, :], in_=pvh[1][0:64, :],
                                         mask=ident)
                linv = lvp.tile([128, SC], f32, tag="linv", name="linv")
                nc.vector.reciprocal_approx_fast(out=linv[:, :], in_=lr[:, :])
                qs = qc * SC
                if split_norm:
                    # narrow chunks so the first epilogue projection (which
                    # needs only 128 OT columns) starts as early as possible
                    for c0 in range(0, SC, 128):
                        nc.vector.tensor_mul(
                            OT_sb[0:64, pair, qs + c0:qs + c0 + 128],
                            pvh[0][0:64, c0:c0 + 128], linv[0:64, c0:c0 + 128])
                        nc.vector.tensor_mul(
                            OT_sb[64:128, pair, qs + c0:qs + c0 + 128],
                            pvh[1][64:128, c0:c0 + 128], linv[64:128, c0:c0 + 128])
                else:
                    nc.vector.tensor_mul(
                        OT_sb[0:64, pair, qs:qs + SC], pvh[0][0:64, :], linv[0:64, :])
                    nc.vector.tensor_mul(
                        OT_sb[64:128, pair, qs:qs + SC], pvh[1][64:128, :], linv[64:128, :])

            # ---- emission schedule ----
            qk = {}
            for jb in range(4):
                for sc in range(4):
                    qk[(jb, sc)] = make_qk_fill(jb, sc)
            vfill = {sb: make_v_fill(sb)[0] for sb in range(NSB)}
            prj = {}
            for sc in range(NSB):
                for nh in range(2):
                    # epilogue chunks (after the last exp) alternate their cast
                    # between DVE and the then-idle ACT engine
                    prj[(sc, nh)] = make_proj_fill(
                        sc, nh, cast_on_scalar=(sc >= 12 and nh == 1))[0]

            # prologue: q+k projections for (pair0, qc0)
            for f in qk[(0, 0)] + qk[(2, 0)]:
                f()

            attention_qc(0, 0, fills=[
                vfill[0], vfill[1], vfill[2], vfill[3],
                *qk[(1, 0)], *qk[(3, 0)]])
            attention_qc(1, 0, fills=[*qk[(0, 1)], *qk[(2, 1)]])
            attention_qc(0, 1, fills=[
                vfill[4], vfill[5], vfill[6], vfill[7],
                *qk[(1, 1)], *qk[(3, 1)]])
            attention_qc(1, 1, fills=[
                *qk[(0, 2)], *qk[(2, 2)],
                prj[(0, 0)], prj[(0, 1)], prj[(1, 0)], prj[(1, 1)]])
            attention_qc(0, 2, fills=[
                vfill[8], vfill[9], vfill[10], vfill[11],
                *qk[(1, 2)], *qk[(3, 2)],
                prj[(2, 0)], prj[(2, 1)], prj[(3, 0)], prj[(3, 1)]])
            attention_qc(1, 2, fills=[
                *qk[(0, 3)], *qk[(2, 3)],
                prj[(4, 0)], prj[(4, 1)], prj[(5, 0)], prj[(5, 1)]])
            attention_qc(0, 3, fills=[
                vfill[12], vfill[13], vfill[14], vfill[15],
                *qk[(1, 3)], *qk[(3, 3)],
                prj[(6, 0)], prj[(6, 1)], prj[(7, 0)], prj[(7, 1)]])
            attention_qc(1, 3, fills=[
                prj[(8, 0)], prj[(8, 1)], prj[(9, 0)], prj[(9, 1)],
                prj[(10, 0)], prj[(10, 1)], prj[(11, 0)], prj[(11, 1)]],
                split_norm=True)
            for sc in range(12, 16):
                prj[(sc, 0)]()
                prj[(sc, 1)]()

    nc.finalize()
    return nc


def _get_nc():
    global _NC
    if _NC is None:
        _NC = _build_nc()
    return _NC


def _prep_in_maps(x, w_qkv, b_qkv):
    x = np.asarray(x, dtype=np.float32)
    w_qkv = np.asarray(w_qkv, dtype=np.float32)
    b_qkv = np.asarray(b_qkv, dtype=np.float32)

    xT_by_batch = [np.ascontiguousarray(x[b].T).astype(ml_dtypes.bfloat16) for b in range(B)]

    mask = np.triu(np.ones((128, 128), dtype=ml_dtypes.bfloat16))  # valid where sq >= sk

    in_maps = []
    for c in range(NCORES):
        b, g = divmod(c, HPC)
        h0 = HPC * g  # first global head for this core
        cq = slice(h0 * D, (h0 + HPC) * D)
        ck = slice(H * D + h0 * D, H * D + (h0 + HPC) * D)

        wqk = np.empty((E, 512), dtype=ml_dtypes.bfloat16)
        wqk[:, 0:256] = w_qkv[:, cq]
        wqk[:, 256:512] = w_qkv[:, ck]

        # b_qkv is zeros by the problem spec (fill: zeros); the device program
        # has no bias path.
        cv = slice(2 * H * D + h0 * D, 2 * H * D + (h0 + HPC) * D)
        wv = np.ascontiguousarray(w_qkv[:, cv]).astype(ml_dtypes.bfloat16)

        in_maps.append({
            "xT": xT_by_batch[b],
            "wqk": wqk,
            "wv": wv,
            "wout": None,  # filled by caller (needs w_out)
            "mask": mask,
        })
    return in_maps


def run(x, w_qkv, b_qkv, w_out, b_out, trace=False, **spmd_kwargs):
    from concourse.bass_utils import run_bass_kernel_spmd

    w_out = np.asarray(w_out, dtype=np.float32)
    b_out = np.asarray(b_out, dtype=np.float32)
    in_maps = _prep_in_maps(x, w_qkv, b_qkv)
    for c in range(NCORES):
        h0 = HPC * (c % HPC)
        in_maps[c]["wout"] = np.ascontiguousarray(w_out[h0 * D:(h0 + HPC) * D, :]).astype(ml_dtypes.bfloat16)

    nc = _get_nc()
    res = run_bass_kernel_spmd(nc, in_maps, core_ids=list(range(NCORES)),
                               trace=trace, **spmd_kwargs)
    out = np.empty((B, S, E), dtype=np.float32)
    for b in range(B):
        acc = res.results[HPC * b]["out_p"].astype(np.float32)
        for i in range(1, HPC):
            acc = acc + res.results[HPC * b + i]["out_p"].astype(np.float32)
        out[b] = acc + b_out
    return out, res


def kernel(x, w_qkv, b_qkv, w_out, b_out):
    out, _ = run(x, w_qkv, b_qkv, w_out, b_out, trace=False)
    return out


# revision 40
# speedup vs baseline: 1.1944x; 1.1818x over previous
"""Multi-head self-attention (B=2, S=2048, E=1024, H=16, D=64, causal) on 8 trn2 cores.

Sharding: tensor-parallel over (batch, head-group). Core c handles batch c//4 and
heads [4*(c%4), 4*(c%4)+4). Each core computes QKV projection for its 4 heads,
causal flash-attention, and a partial output projection (its heads' rows of
w_out). Host sums the 4 partials per batch and adds b_out.

Device math (per core, bf16 matmuls):
  qT/kT [j, s] = (wqk_ext).T @ xT_ext     (j on partitions -> scores need no transpose)
  v_ext [s, j] = xT_ext.T @ wv_ext        (per head: [v|ones] or [ones|v] 128-col block)
  S^T tile [sk, sq] = kT.T-slice @ qT-slice  (two heads row-tiled on the PE)
  P^T = exp(S^T / 8) with causal triangle mask; no max-subtraction needed
  PV: [O^T; L] = v_ext.T @ P^T accumulated over sk chunks; L = softmax denominator
  O^T normalized by 1/L, projected: out_partial = OT.T @ wout_rows

Engine assignment (vs the naive version): ACT (scalar) runs ONLY exp; all
psum->sbuf cast copies run on DVE (vector); causal mask multiplies run on
GpSimd; outputs are written bf16. The out-projection is interleaved into the
attention phase as PE fill work, and the PE/ACT/GPSIMD pipelines are warmed
during the initial DMA window.
"""
import sys

sys.path.insert(0, "/opt/trn_rl_repo")

import ml_dtypes
import numpy as np

import concourse.bacc as bacc
import concourse.mybir as mybir
import concourse.tile as tile



B, S, E = 2, 2048, 1024
H, D = 16, 64
HPC = 4          # heads per core
NCORES = 8
SC = 512         # sq chunk width (scores free dim)
KC = 128         # sk chunk width
NQC = S // SC    # 4 q-chunks
NSB = S // 128   # 16 s-blocks

f32 = mybir.dt.float32
bf16 = mybir.dt.bfloat16

_NC = None


def _build_nc():
    nc = bacc.Bacc(None, target_bir_lowering=False)

    xT = nc.dram_tensor("xT", [E, S], bf16, kind="ExternalInput")
    wqk = nc.dram_tensor("wqk", [E, 512], bf16, kind="ExternalInput")
    wv = nc.dram_tensor("wv", [E, 256], bf16, kind="ExternalInput")
    wout = nc.dram_tensor("wout", [256, E], bf16, kind="ExternalInput")
    mask = nc.dram_tensor("mask", [128, 128], bf16, kind="ExternalInput")
    out_p = nc.dram_tensor("out_p", [S, E], bf16, kind="ExternalOutput")

    with tile.TileContext(nc) as tc:
        with (
            tc.tile_pool(name="big", bufs=1) as big,
            tc.tile_pool(name="ptp", bufs=6) as ptp,
            tc.tile_pool(name="lvp", bufs=2) as lvp,
            tc.tile_pool(name="osb", bufs=3) as osbp,
            tc.tile_pool(name="psA", bufs=2, space="PSUM") as psA,
            tc.tile_pool(name="psB", bufs=1, space="PSUM") as psB,
            tc.tile_pool(name="psF", bufs=2, space="PSUM") as psF,
        ):
            xT_sb = big.tile([128, 8, S], bf16)
            wqk_sb = big.tile([128, 8, 512], bf16)
            wv_sb = big.tile([128, 8, 256], bf16)
            qkT_sb = big.tile([128, 4, S], bf16)
            v_sb = big.tile([128, NSB, 512], bf16)
            OT_sb = big.tile([128, 2, S], bf16)
            wout_sb = big.tile([128, 2, E], bf16)
            mask_sb = big.tile([128, 128], bf16)
            scr_sb = big.tile([128, 128], bf16)   # warmup scratch output

            # ---- engine warmup: ACT table load, GPSIMD library load, PE HAM ----
            # mask is the first (tiny) DMA; everything below depends only on it
            # and executes during the bulk input-DMA window.
            nc.sync.dma_start(out=mask_sb, in_=mask[:, :])
            nc.scalar.activation(
                out=scr_sb[:, 0:8], in_=mask_sb[:, 0:8],
                func=mybir.ActivationFunctionType.Exp, scale=0.125)
            nc.gpsimd.tensor_mul(scr_sb[:, 0:8], mask_sb[:, 0:8], mask_sb[:, 0:8])
            wps = psF.tile([128, SC], f32, name="ps512")
            for _ in range(44):
                nc.tensor.matmul(wps[:, 0:128], mask_sb[:, :], mask_sb[:, :],
                                 start=True, stop=True)

            # ones columns of v_ext: [64:192] and [320:448] within each 512
            # block -- constants, no DMA needed.
            nc.vector.memset(v_sb[:, :, 64:192], 1.0)
            nc.vector.memset(v_sb[:, :, 320:448], 1.0)

            # ---- input DMAs, ordered by first use ----
            # One DMA instruction per logical chunk (descriptor generation on
            # the SP sequencer is ~0.6us per instruction -- many small DMAs
            # starve the prologue). wqk columns: jb0 = q heads 0,1 / jb1 = q
            # heads 2,3 / jb2 = k heads 0,1 / jb3 = k heads 2,3.
            def dma_cols(dst, src_cols):
                nc.sync.dma_start(
                    out=dst, in_=src_cols.rearrange("(b p) c -> p b c", p=128))

            def dma_cols_rows(dst, src_cols, k0, k1):
                nc.sync.dma_start(
                    out=dst[:, k0:k1],
                    in_=src_cols[k0 * 128:k1 * 128, :].rearrange(
                        "(b p) c -> p b c", p=128))

            # first chunk in halves so the prologue matmuls start sooner
            dma_cols_rows(wqk_sb[:, :, 0:128], wqk[:, 0:128], 0, 4)
            dma_cols_rows(xT_sb[:, :, 0:SC], xT[:, 0:SC], 0, 4)
            dma_cols_rows(wqk_sb[:, :, 0:128], wqk[:, 0:128], 4, 8)
            dma_cols_rows(xT_sb[:, :, 0:SC], xT[:, 0:SC], 4, 8)
            dma_cols(wqk_sb[:, :, 256:384], wqk[:, 256:384])
            dma_cols(wv_sb[:, :, :], wv[:, :])
            dma_cols(wqk_sb[:, :, 128:256], wqk[:, 128:256])
            dma_cols(wqk_sb[:, :, 384:512], wqk[:, 384:512])
            for sc4 in range(1, 4):
                dma_cols(xT_sb[:, :, sc4 * SC:(sc4 + 1) * SC],
                         xT[:, sc4 * SC:(sc4 + 1) * SC])
            nc.sync.dma_start(
                out=wout_sb[:, :, :], in_=wout[:, :].rearrange("(b p) c -> p b c", p=128))

            # ---- QKV projection fill units (half = 4 of 8 k-chunks) ----
            def make_qk_fill(jb, sc):
                state = {}

                def half(h):
                    if h == 0:
                        state["ps"] = psF.tile([128, SC], f32, name="ps512")
                    ps = state["ps"]
                    for kc in range(4 * h, 4 * h + 4):
                        nc.tensor.matmul(
                            ps[:, :],
                            wqk_sb[:, kc, jb * 128:(jb + 1) * 128],
                            xT_sb[:, kc, sc * SC:(sc + 1) * SC],
                            start=(kc == 0), stop=(kc == 7))
                    if h == 1:
                        nc.vector.tensor_copy(
                            out=qkT_sb[:, jb, sc * SC:(sc + 1) * SC], in_=ps[:, :])

                return [lambda: half(0), lambda: half(1)]

            def make_v_fill(sb):
                def go():
                    # raw v [128, 256] = xT[:, sb*128:+128].T @ wv; heads h0..h3.
                    # v_ext per head pair: [v_e | ones | ones | v_o].
                    ps = psF.tile([128, SC], f32, name="ps512")[:, 0:256]
                    for kc in range(8):
                        nc.tensor.matmul(
                            ps[:, :],
                            xT_sb[:, kc, sb * 128:(sb + 1) * 128],
                            wv_sb[:, kc, :],
                            start=(kc == 0), stop=(kc == 7))
                    ps3 = ps.rearrange("p (b c) -> p b c", c=128)
                    vs3 = v_sb[:, sb, :].rearrange("p (b c) -> p b c", c=256)
                    nc.vector.tensor_copy(out=vs3[:, :, 0:64], in_=ps3[:, :, 0:64])
                    nc.vector.tensor_copy(out=vs3[:, :, 192:256], in_=ps3[:, :, 64:128])
                return [go]

            def make_proj_fill(sc, nh, cast_on_scalar=False):
                def go():
                    po = psF.tile([128, SC], f32, name="ps512")
                    for p in range(2):
                        nc.tensor.matmul(
                            po[:, :],
                            OT_sb[:, p, sc * 128:(sc + 1) * 128],
                            wout_sb[:, p, SC * nh:SC * nh + SC],
                            start=(p == 0), stop=(p == 1))
                    osb = osbp.tile([128, SC], bf16, name="osb")
                    if cast_on_scalar:
                        nc.scalar.copy(out=osb, in_=po[:, :])
                    else:
                        nc.vector.tensor_copy(out=osb, in_=po[:, :])
                    nc.sync.dma_start(
                        out=out_p[sc * 128:(sc + 1) * 128, SC * nh:SC * nh + SC],
                        in_=osb)
                return [go]

            # ---- attention for one (head pair, q-chunk), with PE fill work ----
            def attention_qc(pair, qc, fills=(), split_norm=False):
                qblk, kblk = pair, 2 + pair
                nkc = 4 * qc + 4
                pv = psB.tile([128, 1024], f32, name="pv")
                pvh = [pv[:, 0:SC], pv[:, SC:1024]]

                def scores_exp(kc):
                    # diagonal tiles (r >= 0): columns < 128*r are causally
                    # invalid -- skip them in the matmul and PV (ragged).
                    r = kc - 4 * qc
                    off = KC * r if r > 0 else 0
                    st = psA.tile([128, 1024], f32, tag="A", name="st")
                    nc.tensor.matmul(
                        st[:, off:SC],
                        qkT_sb[0:64, kblk, kc * KC:(kc + 1) * KC],
                        qkT_sb[0:64, qblk, qc * SC + off:(qc + 1) * SC],
                        start=True, stop=True, tile_position=(0, 0))
                    nc.tensor.matmul(
                        st[:, SC + off:1024],
                        qkT_sb[64:128, kblk, kc * KC:(kc + 1) * KC],
                        qkT_sb[64:128, qblk, qc * SC + off:(qc + 1) * SC],
                        start=True, stop=True, tile_position=(64, 0))
                    pt = ptp.tile([128, 1024], bf16, name="pt")
                    if r < 0:
                        nc.scalar.activation(
                            out=pt[:, :], in_=st[:, :],
                            func=mybir.ActivationFunctionType.Exp, scale=0.125)
                    else:
                        # one wide exp where that costs fewer ACT cycles than two
                        # narrow ones; the gap [SC:SC+off] is junk and never read.
                        if r <= 2:
                            nc.scalar.activation(
                                out=pt[:, off:1024], in_=st[:, off:1024],
                                func=mybir.ActivationFunctionType.Exp, scale=0.125)
                        else:
                            for h2 in range(2):
                                base = SC * h2
                                nc.scalar.activation(
                                    out=pt[:, base + off:base + SC],
                                    in_=st[:, base + off:base + SC],
                                    func=mybir.ActivationFunctionType.Exp, scale=0.125)
                        for h2 in range(2):
                            base = SC * h2
                            tri = pt[:, base + off:base + off + KC]
                            nc.gpsimd.tensor_mul(tri, tri, mask_sb[:, :])
                    return pt

                def pv_step(kc, pt):
                    r = kc - 4 * qc
                    off = KC * r if r > 0 else 0
                    for h2 in range(2):
                        hh = 2 * pair + h2
                        nc.tensor.matmul(
                            pvh[h2][:, off:SC],
                            v_sb[:, kc, 128 * hh:128 * hh + 128],
                            pt[:, SC * h2 + off:SC * h2 + SC],
                            start=(kc == 0), stop=(kc == nkc - 1))

                # pv lags scores by 3 chunks so the pv LDWEIGHTS (which carries
                # the exp-done wait) is always ready and can be pulled ahead.
                lag = min(4, nkc - 1)
                fills = list(fills)
                fi = 0
                pts = {}
                for kc in range(nkc):
                    pts[kc] = scores_exp(kc)
                    if fi < len(fills):
                        fills[fi]()
                        fi += 1
                    if kc >= lag:
                        pv_step(kc - lag, pts.pop(kc - lag))
                while fi < len(fills):
                    fills[fi]()
                    fi += 1
                for kc in range(nkc - lag, nkc):
                    if kc >= 0 and kc in pts:
                        pv_step(kc, pts.pop(kc))

                # normalization: even head [v|ones] -> O rows 0:64 / L rows 64:128
                # of bank0; odd head [ones|v] -> L rows 0:64 / O rows 64:128 of
                # bank1. reciprocal_approx_fast is broken at base_partition != 0,
                # so read full 128 partitions (unused rows produce garbage that
                # is never consumed).
                # shuffle the L rows out of PSUM first (DVE shuffle crossbar;
                # identity mask -- the partition shift is encoded in the AP base
                # partitions), then one narrow reciprocal on SBUF. Keeps the
                # whole normalization chain on one engine queue.
                lr = lvp.tile([128, SC], f32, tag="rec", name="rec")
                ident = list(range(32))
                nc.vector.stream_shuffle(out=lr[0:64, :], in_=pvh[0][64:128, :],
                                         mask=ident)
                nc.vector.stream_shuffle(out=lr[64:128, :], in_=pvh[1][0:64, :],
                                         mask=ident)
                linv = lvp.tile([128, SC], f32, tag="linv", name="linv")
                nc.vector.reciprocal_approx_fast(out=linv[:, :], in_=lr[:, :])
                qs = qc * SC
                if split_norm:
                    # narrow chunks so the first epilogue projection (which
                    # needs only 128 OT columns) starts as early as possible
                    for c0 in range(0, SC, 128):
                        nc.vector.tensor_mul(
                            OT_sb[0:64, pair, qs + c0:qs + c0 + 128],
                            pvh[0][0:64, c0:c0 + 128], linv[0:64, c0:c0 + 128])
                        nc.vector.tensor_mul(
                            OT_sb[64:128, pair, qs + c0:qs + c0 + 128],
                            pvh[1][64:128, c0:c0 + 128], linv[64:128, c0:c0 + 128])
                else:
                    nc.vector.tensor_mul(
                        OT_sb[0:64, pair, qs:qs + SC], pvh[0][0:64, :], linv[0:64, :])
                    nc.vector.tensor_mul(
                        OT_sb[64:128, pair, qs:qs + SC], pvh[1][64:128, :], linv[64:128, :])

            # ---- emission schedule ----
            qk = {}
            for jb in range(4):
                for sc in range(4):
                    qk[(jb, sc)] = make_qk_fill(jb, sc)
            vfill = {sb: make_v_fill(sb)[0] for sb in range(NSB)}
            prj = {}
            for sc in range(NSB):
                for nh in range(2):
                    # epilogue chunks (after the last exp) alternate their cast
                    # between DVE and the then-idle ACT engine
                    prj[(sc, nh)] = make_proj_fill(
                        sc, nh, cast_on_scalar=(sc >= 12 and nh == 1))[0]

            # prologue: q+k projections for (pair0, qc0)
            for f in qk[(0, 0)] + qk[(2, 0)]:
                f()

            attention_qc(0, 0, fills=[
                vfill[0], vfill[1], vfill[2], vfill[3],
                *qk[(1, 0)], *qk[(3, 0)]])
            attention_qc(1, 0, fills=[*qk[(0, 1)], *qk[(2, 1)]])
            attention_qc(0, 1, fills=[
                vfill[4], vfill[5], vfill[6], vfill[7],
                *qk[(1, 1)], *qk[(3, 1)]])
            attention_qc(1, 1, fills=[
                *qk[(0, 2)], *qk[(2, 2)],
                prj[(0, 0)], prj[(0, 1)], prj[(1, 0)], prj[(1, 1)]])
            attention_qc(0, 2, fills=[
                vfill[8], vfill[9], vfill[10], vfill[11],
                *qk[(1, 2)], *qk[(3, 2)],
                prj[(2, 0)], prj[(2, 1)], prj[(3, 0)], prj[(3, 1)]])
            attention_qc(1, 2, fills=[
                *qk[(0, 3)], *qk[(2, 3)],
                prj[(4, 0)], prj[(4, 1)], prj[(5, 0)], prj[(5, 1)]])
            attention_qc(0, 3, fills=[
                vfill[12], vfill[13], vfill[14], vfill[15],
                *qk[(1, 3)], *qk[(3, 3)],
                prj[(6, 0)], prj[(6, 1)], prj[(7, 0)], prj[(7, 1)]])
            attention_qc(1, 3, fills=[
                prj[(8, 0)], prj[(8, 1)], prj[(9, 0)], prj[(9, 1)],
                prj[(10, 0)], prj[(10, 1)], prj[(11, 0)], prj[(11, 1)]],
                split_norm=True)
            for sc in range(12, 16):
                prj[(sc, 0)]()
                prj[(sc, 1)]()

    nc.finalize()
    return nc


def _get_nc():
    global _NC
    if _NC is None:
        _NC = _build_nc()
    return _NC


def _prep_in_maps(x, w_qkv, b_qkv):
    x = np.asarray(x, dtype=np.float32)
    w_qkv = np.asarray(w_qkv, dtype=np.float32)
    b_qkv = np.asarray(b_qkv, dtype=np.float32)

    xT_by_batch = [np.ascontiguousarray(x[b].T).astype(ml_dtypes.bfloat16) for b in range(B)]

    mask = np.triu(np.ones((128, 128), dtype=ml_dtypes.bfloat16))  # valid where sq >= sk

    in_maps = []
    for c in range(NCORES):
        b, g = divmod(c, HPC)
        h0 = HPC * g  # first global head for this core
        cq = slice(h0 * D, (h0 + HPC) * D)
        ck = slice(H * D + h0 * D, H * D + (h0 + HPC) * D)

        wqk = np.empty((E, 512), dtype=ml_dtypes.bfloat16)
        wqk[:, 0:256] = w_qkv[:, cq]
        wqk[:, 256:512] = w_qkv[:, ck]

        # b_qkv is zeros by the problem spec (fill: zeros); the device program
        # has no bias path.
        cv = slice(2 * H * D + h0 * D, 2 * H * D + (h0 + HPC) * D)
        wv = np.ascontiguousarray(w_qkv[:, cv]).astype(ml_dtypes.bfloat16)

        in_maps.append({
            "xT": xT_by_batch[b],
            "wqk": wqk,
            "wv": wv,
            "wout": None,  # filled by caller (needs w_out)
            "mask": mask,
        })
    return in_maps


def run(x, w_qkv, b_qkv, w_out, b_out, trace=False, **spmd_kwargs):
    from concourse.bass_utils import run_bass_kernel_spmd

    w_out = np.asarray(w_out, dtype=np.float32)
    b_out = np.asarray(b_out, dtype=np.float32)
    in_maps = _prep_in_maps(x, w_qkv, b_qkv)
    for c in range(NCORES):
        h0 = HPC * (c % HPC)
        in_maps[c]["wout"] = np.ascontiguousarray(w_out[h0 * D:(h0 + HPC) * D, :]).astype(ml_dtypes.bfloat16)

    nc = _get_nc()
    res = run_bass_kernel_spmd(nc, in_maps, core_ids=list(range(NCORES)),
                               trace=trace, **spmd_kwargs)
    out = np.empty((B, S, E), dtype=np.float32)
    for b in range(B):
        acc = res.results[HPC * b]["out_p"].astype(np.float32)
        for i in range(1, HPC):
            acc = acc + res.results[HPC * b + i]["out_p"].astype(np.float32)
        out[b] = acc + b_out
    return out, res


def kernel(x, w_qkv, b_qkv, w_out, b_out):
    out, _ = run(x, w_qkv, b_qkv, w_out, b_out, trace=False)
    return out
